# revision 18
# baseline (speedup 1.0000x reference)
"""Trainium2 Bass kernel for nn_CycleGNN (8-step projected-direction solver).

Contract: kernel(**inputs) takes the FULL unsharded numpy inputs (keyed as in
setup_inputs()) and returns the full output (preds, labels), each
[131072, 8] float32.  Internally shards the 64 graphs across 8 NeuronCores
(8 graphs per core, graphs never interact -> no collectives), runs a Tile
kernel via run_bass_kernel_spmd, and re-assembles on the host.

Device-side layout notes (per core, 8 graphs, 16384 nodes):
 - per-node state is "p-major banded" [128, 128]: tile[p, c] = v[p*128 + c];
   graph g owns partitions [16g, 16g+16).
 - P (bf16) stays resident in SBUF ([128, 8, 16, 512]) and is the stationary
   operand of column-form einsum1 matmuls (out df column [128,1] per
   (graph, f-chunk), moving operand = one d column).
 - PT (bf16) streams from DRAM each step ([128, 2048] per (g, f-chunk)) and
   is the stationary operand of column-form einsum2 (out y column [128,1]
   per (graph, n-chunk), moving = one df column).
 - column-form outputs land as PSUM "colmats" [128, 128] which one PE
   transpose converts back to banded layout - no partition-scatter DMAs.
 - the MLP runs hid-partition: stationary [W1[:64]; b1; W1[64]] over a
   [66, NPC] moving operand (features + ones row + xs row); the xs row is
   refreshed each step by 8 small partition-gather DMAs from banded xs.
"""

import numpy as np
import ml_dtypes

import bass_rust
import concourse.bass as bass
import concourse.tile as tile
from concourse import mybir
from concourse.bass_utils import run_bass_kernel_spmd
from concourse.masks import make_identity

F32 = mybir.dt.float32
BF16 = mybir.dt.bfloat16
BF = ml_dtypes.bfloat16

B = 64          # graphs
NMAX = 2048     # nodes per graph (equal-size, sorted vals_batch)
F = 512         # projection basis dim
HID = 128
NFEAT = 64
NUM_STEPS = 8
STEP_ALPHA = 5.0
NCORES = 8
GPC = B // NCORES            # graphs per core = 8
NPC = GPC * NMAX             # nodes per core = 16384
NCH = NMAX // 128            # n-chunks per graph = 16
FCH = F // 128               # f-chunks = 4

AX = mybir.AxisListType
OP = mybir.AluOpType
ACT = mybir.ActivationFunctionType

_COMPILED = {}


def _split_sync_waits(nc, maxw=1):
    """Walrus in this container accepts at most one sync wait per
    instruction; split extra waits into preceding engine-local NoOps."""
    ctr = 0
    for f in nc.m.functions:
        for bb in f.blocks:
            insts = bb.instructions
            out = []
            changed = False
            for ins in insts:
                si = ins.sync_info
                waits = list(si.on_wait) if si is not None else []
                if len(waits) > maxw:
                    reg_waits = [w for w in waits if w.wait_reg is not None]
                    imm_waits = [w for w in waits if w.wait_reg is None]
                    nkeep = max(0, maxw - len(reg_waits))
                    keep = imm_waits[:nkeep]
                    extra = imm_waits[nkeep:]
                    for i in range(0, len(extra), maxw):
                        ctr += 1
                        nop = mybir.InstNoOp(name=f"wsplit-{ctr}", ins=[], outs=[])
                        nop.engine = ins.engine
                        nop.sync_info = bass_rust.SyncInfo(
                            on_wait=extra[i : i + maxw], on_update=[]
                        )
                        out.append(nop)
                    ins.sync_info = bass_rust.SyncInfo(
                        on_wait=reg_waits + keep, on_update=list(si.on_update)
                    )
                    changed = True
                out.append(ins)
            if changed:
                bb.instructions = out
    return ctr


def _tau_schedule():
    taus = []
    tau = 0.01
    for _ in range(NUM_STEPS):
        taus.append(tau)
        tau = max(tau * 0.5, 1e-5)
    return taus


def build_nc(debug=False, num_steps=NUM_STEPS, skip=(), reps=1):
    nc = bass.Bass()

    # ---------------- I/O ----------------
    P_d = nc.declare_dram_parameter("P", [128, GPC, NCH, F], BF16, isOutput=False)
    PT_d = nc.declare_dram_parameter("PT", [GPC, FCH, 128, NMAX], BF16, isOutput=False)
    nfTp_d = nc.declare_dram_parameter("nfTp", [NFEAT + 2, NPC], BF16, isOutput=False)
    xs0_d = nc.declare_dram_parameter("xs0", [128, 128], F32, isOutput=False)
    xsol_d = nc.declare_dram_parameter("xsol", [128, 128], F32, isOutput=False)
    w1a_d = nc.declare_dram_parameter("w1a", [NFEAT + 2, HID], BF16, isOutput=False)
    w2_d = nc.declare_dram_parameter("w2", [HID, 1], BF16, isOutput=False)
    b2_d = nc.declare_dram_parameter("b2", [1, 1], F32, isOutput=False)
    seg_d = nc.declare_dram_parameter("seg", [128, 128], F32, isOutput=False)
    seg8_d = nc.declare_dram_parameter("seg8", [GPC, 128], F32, isOutput=False)

    preds_o = nc.declare_dram_parameter("preds", [NUM_STEPS, NPC], F32, isOutput=True)
    labels_o = nc.declare_dram_parameter("labels", [NUM_STEPS, NPC], F32, isOutput=True)
    if debug:
        dbg_alpha = nc.declare_dram_parameter("dbg_alpha", [NUM_STEPS, GPC], F32, isOutput=True)
        dbg_df0 = nc.declare_dram_parameter("dbg_df0", [128, 32], F32, isOutput=True)
        dbg_y0 = nc.declare_dram_parameter("dbg_y0", [128, 128], F32, isOutput=True)
        dbg_d0 = nc.declare_dram_parameter("dbg_d0", [128, 128], F32, isOutput=True)

    taus = _tau_schedule()

    with tile.TileContext(nc) as tc:
        with (
            tc.tile_pool(name="res", bufs=1) as res,            # resident singles
            tc.tile_pool(name="ptp", bufs=8) as ptp,            # PT stream chunks
            tc.tile_pool(name="hp", bufs=3) as hp,              # relu'd hidden chunks
            tc.tile_pool(name="smt", bufs=1) as smt,            # small temps / state
            tc.tile_pool(name="mlp_ps", bufs=2, space="PSUM") as mlp_ps,
            tc.tile_pool(name="cm_ps", bufs=1, space="PSUM") as cm_ps,    # colmats (df/pred/y)
            tc.tile_pool(name="tr_ps", bufs=1, space="PSUM") as tr_ps,    # transposes
            tc.tile_pool(name="sg_ps", bufs=1, space="PSUM") as sg_ps,    # seg matmuls / micro
        ):
            # ---------------- constants / residents ----------------
            identf = res.tile([128, 128], F32, tag="identf")
            make_identity(nc, identf)
            identb = res.tile([128, 128], BF16, tag="identb")
            make_identity(nc, identb)
            onesf = res.tile([128, 1], F32, tag="onesf")
            nc.vector.memset(onesf, 1.0)

            seg = res.tile([128, 128], F32, tag="seg")
            nc.sync.dma_start(out=seg, in_=seg_d[:])
            seg8 = res.tile([GPC, 128], F32, tag="seg8")
            nc.sync.dma_start(out=seg8, in_=seg8_d[:])

            w1a = res.tile([NFEAT + 2, HID], BF16, tag="w1a")
            nc.sync.dma_start(out=w1a, in_=w1a_d[:])
            w2 = res.tile([HID, 1], BF16, tag="w2")
            nc.sync.dma_start(out=w2, in_=w2_d[:])
            b2c = res.tile([128, 1], F32, tag="b2c")
            nc.sync.dma_start(
                out=b2c,
                in_=bass.AP(tensor=b2_d, offset=0, ap=[[0, 128], [1, 1]]),
            )

            # state (small; load before the big residents so step 0 can start)
            xs = res.tile([128, 128], F32, tag="xs")
            nc.sync.dma_start(out=xs, in_=xs0_d[:])
            xsol = res.tile([128, 128], F32, tag="xsol")
            nc.sync.dma_start(out=xsol, in_=xsol_d[:])

            # MLP moving operand: rows 0..63 features, row 64 ones (folds b1
            # via the extra row of w1a), row 65 = xs (refreshed per step);
            # split the load so MLP chunks can start while later pieces
            # stream.
            nfTp = res.tile([NFEAT + 2, NPC], BF16, tag="nfTp")
            for q in range(4):
                nc.sync.dma_start(
                    out=nfTp[:, 4096 * q : 4096 * (q + 1)],
                    in_=nfTp_d[:, 4096 * q : 4096 * (q + 1)],
                )

            # big resident P (bf16); split by graph across DMAs
            sbP = res.tile([128, GPC, NCH, F], BF16, tag="sbP")
            for g in range(GPC):
                nc.scalar.dma_start(out=sbP[:, g], in_=P_d[:, g])

            for rep in range(reps):
              if rep > 0:
                # re-run the whole workload on the same inputs (throughput
                # measurement); reset the solver state
                nc.sync.dma_start(out=xs, in_=xs0_d[:])
              for s in range(num_steps):
                tau = taus[s]

                # ---- A: MLP  h = relu(W1f^T nf + b1 + W1x^T xs) ----
                xs_bf = smt.tile([128, 128], BF16, tag="xs_bf")
                nc.vector.tensor_copy(xs_bf, xs)
                # refresh the xs row of the moving operand (8 small
                # partition-gather DMAs; issued from gpsimd = cheap)
                for g in range(GPC):
                    nc.gpsimd.dma_start(
                        out=nfTp[
                            NFEAT + 1 : NFEAT + 2, 2048 * g : 2048 * (g + 1)
                        ].rearrange("o (p c) -> o p c", p=16),
                        in_=xs_bf[16 * g : 16 * g + 16, :],
                    )
                # recv = 1/(xs+tau): independent of pred, compute early
                recv = smt.tile([128, 128], F32, tag="recv")
                nc.vector.tensor_scalar_add(recv, xs, float(tau))
                nc.vector.reciprocal(recv, recv)
                nc.vector.tensor_scalar(
                    out=recv, in0=recv, scalar1=float(3.0 * tau), scalar2=None,
                    op0=OP.mult,
                )

                if "dchain" not in skip:
                    diff = smt.tile([128, 128], F32, tag="diff")
                    nc.vector.tensor_sub(diff, xsol, xs)
                    adiff = smt.tile([128, 128], F32, tag="adiff", bufs=1)
                    nc.vector.scalar_tensor_tensor(
                        out=adiff, in0=diff, scalar=-1.0, in1=diff,
                        op0=OP.mult, op1=OP.max,
                    )
                    lab_part = smt.tile([128, 1], F32, tag="lab_part")
                    nc.vector.tensor_reduce(
                        out=lab_part, in_=adiff, axis=AX.X, op=OP.add
                    )
                    ls_ps = sg_ps.tile([128, 1], F32, tag="sg")
                    nc.tensor.matmul(ls_ps, seg, lab_part, start=True, stop=True)
                    lscale = smt.tile([128, 1], F32, tag="lscale")
                    nc.vector.tensor_scalar_max(lscale, ls_ps, 1e-8)
                    nc.vector.reciprocal(lscale, lscale)
                    label = smt.tile([128, 128], F32, tag="label")
                    nc.vector.tensor_scalar(
                        out=label, in0=diff, scalar1=lscale, scalar2=None, op0=OP.mult
                    )
                    nc.sync.dma_start(
                        out=labels_o[s].rearrange("(p c) -> p c", p=128), in_=label
                    )

                pred_ps = cm_ps.tile([128, 128], F32, tag="pred_ps")
                NWM = 64 if "mlpmm" in skip else 512
                for j in range(32):
                    hpsum = mlp_ps.tile([128, 512], F32, tag="hpsum")
                    nc.tensor.matmul(
                        hpsum[:, 0:NWM],
                        w1a,
                        nfTp[:, 512 * j : 512 * j + NWM],
                        start=True,
                        stop=True,
                    )
                    hpos = hp.tile([128, 512], BF16, tag="hpos")
                    NWR = 64 if "relu" in skip else 512
                    if j % 2 == 0:
                        nc.scalar.activation(
                            out=hpos[:, 0:NWR], in_=hpsum[:, 0:NWR], func=ACT.Relu
                        )
                    else:
                        nc.vector.tensor_scalar(
                            out=hpos[:, 0:NWR], in0=hpsum[:, 0:NWR],
                            scalar1=0.0, scalar2=None, op0=OP.max,
                        )
                    if NWR < 512:
                        nc.vector.tensor_copy(hpos[:, NWR:512], hpos[:, 0:512 - NWR])
                    # W2 column-form: one pred column per 128-node chunk
                    for t in range(4):
                        nc.tensor.matmul(
                            pred_ps[:, 4 * j + t : 4 * j + t + 1],
                            hpos[:, 128 * t : 128 * (t + 1)],
                            w2,
                            start=True,
                            stop=True,
                        )
                # colmat -> banded: evac (+b2) then one PE transpose
                pred_cm = smt.tile([128, 128], BF16, tag="pred_cm")
                nc.vector.tensor_scalar(
                    out=pred_cm, in0=pred_ps, scalar1=b2c, scalar2=None, op0=OP.add
                )
                predT_ps = tr_ps.tile([128, 128], BF16, tag="tr", name="predT_ps")
                nc.tensor.transpose(predT_ps, pred_cm, identb)
                pred = smt.tile([128, 128], BF16, tag="pred", bufs=2)
                nc.vector.tensor_copy(pred, predT_ps)

                # ---- B: l1norm scales + labels + direction d ----
                if "dchain" in skip:
                    diff0 = smt.tile([128, 128], F32, tag="diff", bufs=1)
                    nc.vector.tensor_sub(diff0, xsol, xs)
                    nc.sync.dma_start(
                        out=labels_o[s].rearrange("(p c) -> p c", p=128), in_=diff0
                    )
                    nc.gpsimd.dma_start(
                        out=preds_o[s].rearrange("(p c) -> p c", p=128), in_=pred
                    )
                    d_c = smt.tile([128, 128], BF16, tag="d_c")
                    nc.vector.memset(d_c, 0.01)
                else:
                    # scale-folded direction: d' = pred + s * 3tau/(xs+tau)
                    # (s = sum_g |pred|; the true d = d'/s, and the division
                    # is deferred into the line-search alpha, which is
                    # scale-invariant up to the min-with-STEP_ALPHA).
                    # s chain runs on the colmat, in parallel with the pred
                    # transpose.
                    apred = smt.tile([128, 128], F32, tag="apred", bufs=1)
                    nc.vector.scalar_tensor_tensor(
                        out=apred, in0=pred_cm, scalar=-1.0, in1=pred_cm,
                        op0=OP.mult, op1=OP.max,
                    )
                    part8 = smt.tile([128, GPC], F32, tag="part8")
                    nc.vector.tensor_reduce(
                        out=part8,
                        in_=apred.rearrange("p (g c) -> p g c", g=GPC),
                        axis=AX.X,
                        op=OP.add,
                    )
                    s8c_ps = sg_ps.tile([128, 1], F32, tag="sg")
                    nc.tensor.matmul(
                        s8c_ps[0:GPC, :], part8, onesf, start=True, stop=True
                    )
                    s8c = smt.tile([GPC, 1], F32, tag="s8c")
                    nc.vector.tensor_scalar_max(s8c, s8c_ps[0:GPC, :], 1e-8)
                    scol_ps = sg_ps.tile([128, 1], F32, tag="sg")
                    nc.tensor.matmul(scol_ps, seg8, s8c, start=True, stop=True)
                    s_col = smt.tile([128, 1], F32, tag="s_col")
                    nc.vector.tensor_copy(s_col, scol_ps)
                    # s as a row for the alpha stage (off the d critical path)
                    s8r_ps = sg_ps.tile([1, 128], F32, tag="sgw", name="s8r_ps")
                    nc.tensor.transpose(s8r_ps[:, 0:GPC], s8c, identf[0:GPC, 0:GPC])
                    s8row = smt.tile([1, GPC], F32, tag="s8row")
                    nc.vector.tensor_copy(s8row, s8r_ps[:, 0:GPC])

                    nc.gpsimd.dma_start(
                        out=preds_o[s].rearrange("(p c) -> p c", p=128), in_=pred
                    )
                    d_bf = smt.tile([128, 128], BF16, tag="d_bf")
                    nc.vector.scalar_tensor_tensor(
                        out=d_bf, in0=recv, scalar=s_col, in1=pred,
                        op0=OP.mult, op1=OP.add,
                    )
                    if debug and s == 0:
                        d_dbg = smt.tile([128, 128], F32, tag="d_dbg")
                        nc.vector.tensor_copy(d_dbg, d_bf)
                        nc.sync.dma_start(out=dbg_d0[:], in_=d_dbg)
                    dct_ps = tr_ps.tile([128, 128], BF16, tag="tr", name="dct_ps")
                    nc.tensor.transpose(dct_ps, d_bf, identb)
                    d_c = smt.tile([128, 128], BF16, tag="d_c")
                    nc.vector.tensor_copy(d_c, dct_ps)

                # ---- C: einsum1 (column form)  df[g,f] = sum_n P[g,n,f] d[g,n]
                df_ps = cm_ps.tile([128, 32], F32, tag="df_ps")
                NK1 = 2 if "e1mm" in skip else NCH
                for g in range(GPC):
                    for j in range(FCH):
                        for k in range(NK1):
                            nc.tensor.matmul(
                                df_ps[:, 4 * g + j : 4 * g + j + 1],
                                sbP[:, g, k, 128 * j : 128 * (j + 1)],
                                d_c[:, 16 * g + k : 16 * g + k + 1],
                                start=(k == 0),
                                stop=(k == NK1 - 1),
                            )
                dfb = smt.tile([128, 32], BF16, tag="dfb")
                nc.vector.tensor_copy(dfb, df_ps)
                if debug and s == 0:
                    dfb32 = smt.tile([128, 32], F32, tag="dfb32")
                    nc.vector.tensor_copy(dfb32, df_ps)
                    nc.sync.dma_start(out=dbg_df0[:], in_=dfb32)

                # ---- D: einsum2 (column form)  y[g,n] = sum_f PT[g,f,n] df[g,f]
                y_ps = cm_ps.tile([128, 128], F32, tag="y_ps")
                NK2 = 2 if "e2mm" in skip else NCH
                for g in range(GPC):
                    ptts = []
                    for j in range(FCH):
                        ptt = ptp.tile([128, NMAX], BF16, tag="ptt", name="ptt")
                        if "ptdma" not in skip:
                            nc.sync.dma_start(out=ptt, in_=PT_d[g, j])
                        else:
                            nc.sync.dma_start(out=ptt[:, 0:256], in_=PT_d[g, j, :, 0:256])
                        ptts.append(ptt)
                    for k in range(NK2):
                        for j in range(FCH):
                            nc.tensor.matmul(
                                y_ps[:, 16 * g + k : 16 * g + k + 1],
                                ptts[j][:, 128 * k : 128 * (k + 1)],
                                dfb[:, 4 * g + j : 4 * g + j + 1],
                                start=(j == 0),
                                stop=(j == FCH - 1),
                            )
                y_cm = smt.tile([128, 128], BF16, tag="y_cm")
                nc.vector.tensor_copy(y_cm, y_ps)
                yT_ps = tr_ps.tile([128, 128], BF16, tag="tr", name="yT_ps")
                nc.tensor.transpose(yT_ps, y_cm, identb)
                y_pm = smt.tile([128, 128], BF16, tag="y_pm", bufs=2)
                nc.vector.tensor_copy(y_pm, yT_ps)
                if debug and s == 0:
                    y_dbg = smt.tile([128, 128], F32, tag="y_dbg")
                    nc.vector.tensor_copy(y_dbg, y_pm)
                    nc.sync.dma_start(out=dbg_y0[:], in_=y_dbg)

                # ---- E: line search + state update ----
                if "alpha" in skip:
                    acol0 = smt.tile([128, 1], F32, tag="acol")
                    nc.vector.memset(acol0, 0.05)
                    ay0 = smt.tile([128, 128], F32, tag="ay", bufs=1)
                    nc.vector.tensor_scalar(
                        out=ay0, in0=y_pm, scalar1=acol0, scalar2=None, op0=OP.mult
                    )
                    nc.vector.tensor_add(xs, xs, ay0)
                    continue
                q = smt.tile([128, 128], F32, tag="q", bufs=1)
                nc.vector.tensor_scalar(
                    out=q, in0=y_pm, scalar1=-1.0, scalar2=1e-30,
                    op0=OP.mult, op1=OP.max,
                )
                nc.vector.reciprocal(q, q)
                stp = smt.tile([128, 128], F32, tag="stp", bufs=1)
                nc.vector.tensor_mul(stp, xs, q)
                smin = smt.tile([128, 1], F32, tag="smin")
                nc.vector.tensor_reduce(out=smin, in_=stp, axis=AX.X, op=OP.min)
                # per-graph min: transpose -> row -> seg-min -> back to column
                smin_ps = sg_ps.tile([1, 128], F32, tag="sgw")
                nc.tensor.transpose(smin_ps, smin, identf)
                smin_row = smt.tile([1, 128], F32, tag="smin_row")
                nc.vector.tensor_copy(smin_row, smin_ps)
                amin_row = smt.tile([1, GPC], F32, tag="amin_row")
                nc.vector.tensor_reduce(
                    out=amin_row,
                    in_=smin_row.rearrange("o (g b) -> o g b", g=GPC),
                    axis=AX.X,
                    op=OP.min,
                )
                if "dchain" not in skip:
                    nc.vector.tensor_mul(amin_row, amin_row, s8row)
                nc.vector.tensor_scalar(
                    out=amin_row, in0=amin_row, scalar1=float(STEP_ALPHA),
                    scalar2=0.995, op0=OP.min, op1=OP.mult,
                )
                if "dchain" not in skip:
                    s8inv = smt.tile([1, GPC], F32, tag="s8inv")
                    nc.vector.reciprocal(s8inv, s8row)
                    nc.vector.tensor_mul(amin_row, amin_row, s8inv)
                if debug:
                    nc.sync.dma_start(out=dbg_alpha[s], in_=amin_row)
                a8_ps = sg_ps.tile([GPC, 1], F32, tag="sg")
                nc.tensor.transpose(a8_ps, amin_row, identf[0:1, 0:1])
                a8 = smt.tile([GPC, 1], F32, tag="a8")
                nc.vector.tensor_copy(a8, a8_ps)
                acol_ps = sg_ps.tile([128, 1], F32, tag="sg")
                nc.tensor.matmul(acol_ps, seg8, a8, start=True, stop=True)
                acol = smt.tile([128, 1], F32, tag="acol")
                nc.vector.tensor_copy(acol, acol_ps)

                ay = smt.tile([128, 128], F32, tag="ay", bufs=1)
                nc.vector.tensor_scalar(
                    out=ay, in0=y_pm, scalar1=acol, scalar2=None, op0=OP.mult
                )
                nc.vector.tensor_add(xs, xs, ay)

    _split_sync_waits(nc, maxw=1)
    return nc


def _seg_mats():
    seg = np.zeros((128, 128), np.float32)
    for g in range(GPC):
        seg[16 * g : 16 * g + 16, 16 * g : 16 * g + 16] = 1.0
    seg8 = np.zeros((GPC, 128), np.float32)
    for g in range(GPC):
        seg8[g, 16 * g : 16 * g + 16] = 1.0
    return seg, seg8


def _prep_core_inputs(core, proj, x_start, x_solution, node_feat, W1, b1, W2, b2):
    g0 = core * GPC
    n0 = core * NPC
    Pc = proj[g0 : g0 + GPC]  # [8, 2048, 512] f32
    P_bf = np.ascontiguousarray(
        Pc.reshape(GPC, NCH, 128, F).transpose(2, 0, 1, 3)
    ).astype(BF)
    PT_bf = np.ascontiguousarray(Pc.transpose(0, 2, 1)).reshape(
        GPC, FCH, 128, NMAX
    ).astype(BF)
    nfTp = np.empty((NFEAT + 2, NPC), BF)
    nfTp[0:NFEAT] = np.ascontiguousarray(node_feat[n0 : n0 + NPC].T).astype(BF)
    nfTp[NFEAT] = np.ones((NPC,), BF)
    nfTp[NFEAT + 1] = np.zeros((NPC,), BF)  # xs row, written on device
    w1a = np.empty((NFEAT + 2, HID), BF)
    w1a[0:NFEAT] = W1[0:NFEAT].astype(BF)
    w1a[NFEAT] = b1.astype(BF)
    w1a[NFEAT + 1] = W1[NFEAT].astype(BF)
    seg, seg8 = _seg_mats()
    return {
        "P": P_bf,
        "PT": PT_bf,
        "nfTp": nfTp,
        "xs0": x_start[n0 : n0 + NPC].reshape(128, 128).astype(np.float32),
        "xsol": x_solution[n0 : n0 + NPC].reshape(128, 128).astype(np.float32),
        "w1a": w1a,
        "w2": W2.reshape(HID, 1).astype(BF),
        "b2": b2.reshape(1, 1).astype(np.float32),
        "seg": seg,
        "seg8": seg8,
    }


def _numpy_fallback(x_start, x_solution, node_feat, proj_matrix, W1, b1, W2, b2, batch):
    """General (ragged) reference implementation in numpy, used only if
    vals_batch is not the expected equal-size pattern."""
    nb = proj_matrix.shape[0]
    batch = batch.astype(np.int64)
    counts = np.bincount(batch, minlength=nb)
    offsets = np.cumsum(counts) - counts
    pos = np.arange(batch.shape[0]) - offsets[batch]

    def l1norm(x):
        s = np.zeros(nb, x.dtype)
        np.add.at(s, batch, np.abs(x))
        return x / np.clip(s, 1e-8, None)[batch]

    def to_dense(x):
        dense = np.zeros((nb, NMAX), x.dtype)
        m = pos < NMAX
        dense[batch[m], pos[m]] = x[m]
        return dense

    def line_search(x, dvec):
        neg = dvec < 0
        step = np.where(neg, x / np.where(neg, -dvec, 1.0), STEP_ALPHA)
        a = np.full(nb, np.inf, step.dtype)
        np.minimum.at(a, batch, step)
        return np.minimum(a, STEP_ALPHA)[batch]

    def gnn(x):
        h = np.concatenate([node_feat, x[:, None]], axis=-1)
        h = np.maximum(h @ W1 + b1, 0.0)
        return (h @ W2 + b2)[:, 0]

    tau = 0.01
    xs = x_start.astype(np.float32)
    preds, labels = [], []
    for _ in range(NUM_STEPS):
        pred = gnn(xs)
        preds.append(pred)
        labels.append(l1norm(x_solution - xs))
        p = l1norm(pred)
        direction = p + 3.0 * tau / (xs + tau)
        tau = max(tau * 0.5, 1e-5)
        d_dense = to_dense(direction)
        df = np.einsum("bnf,bn->bf", proj_matrix, d_dense)
        proj_dense = np.einsum("bnf,bf->bn", proj_matrix, df)
        proj_flat = proj_dense[batch, np.minimum(pos, NMAX - 1)]
        proj_flat = np.where(pos < NMAX, proj_flat, 0.0)
        alpha = line_search(xs, proj_flat) * 0.995
        xs = xs + alpha * proj_flat
    return np.stack(preds, 1).astype(np.float32), np.stack(labels, 1).astype(np.float32)


def run_on_hw(inputs_list, debug=False):
    key = "dbg" if debug else "plain"
    if key not in _COMPILED:
        _COMPILED[key] = build_nc(debug=debug)
    nc = _COMPILED[key]
    return run_bass_kernel_spmd(nc, inputs_list, list(range(NCORES))).results


def kernel(x_start, x_solution, node_feat, proj_matrix, W1, b1, W2, b2, vals_batch):
    expected = np.repeat(np.arange(B, dtype=np.int64), NMAX)
    vb = np.asarray(vals_batch)
    if vb.shape != expected.shape or not np.array_equal(
        vb.astype(np.int64), expected
    ):
        return _numpy_fallback(
            np.asarray(x_start, np.float32),
            np.asarray(x_solution, np.float32),
            np.asarray(node_feat, np.float32),
            np.asarray(proj_matrix, np.float32),
            np.asarray(W1, np.float32),
            np.asarray(b1, np.float32),
            np.asarray(W2, np.float32),
            np.asarray(b2, np.float32),
            vb,
        )

    x_start = np.asarray(x_start, np.float32)
    x_solution = np.asarray(x_solution, np.float32)
    node_feat = np.asarray(node_feat, np.float32)
    proj_matrix = np.asarray(proj_matrix, np.float32)
    W1 = np.asarray(W1, np.float32)
    b1 = np.asarray(b1, np.float32)
    W2 = np.asarray(W2, np.float32)
    b2 = np.asarray(b2, np.float32)

    ins = [
        _prep_core_inputs(c, proj_matrix, x_start, x_solution, node_feat, W1, b1, W2, b2)
        for c in range(NCORES)
    ]
    results = run_on_hw(ins)
    preds = np.concatenate(
        [results[c]["preds"].T for c in range(NCORES)], axis=0
    ).astype(np.float32)
    labels = np.concatenate(
        [results[c]["labels"].T for c in range(NCORES)], axis=0
    ).astype(np.float32)
    return preds, labels


# revision 24
# speedup vs baseline: 1.0068x; 1.0068x over previous
"""Trainium2 Bass kernel for nn_CycleGNN (8-step projected-direction solver).

Contract: kernel(**inputs) takes the FULL unsharded numpy inputs (keyed as in
setup_inputs()) and returns the full output (preds, labels), each
[131072, 8] float32.  Internally shards the 64 graphs across 8 NeuronCores
(8 graphs per core, graphs never interact -> no collectives), runs a Tile
kernel via run_bass_kernel_spmd, and re-assembles on the host.

Device-side layout notes (per core, 8 graphs, 16384 nodes):
 - per-node state is "p-major banded" [128, 128]: tile[p, c] = v[p*128 + c];
   graph g owns partitions [16g, 16g+16).
 - P (bf16) stays resident in SBUF ([128, 8, 16, 512]) and is the stationary
   operand of column-form einsum1 matmuls (out df column [128,1] per
   (graph, f-chunk), moving operand = one d column).
 - PT (bf16) streams from DRAM each step ([128, 2048] per (g, f-chunk)) and
   is the stationary operand of column-form einsum2 (out y column [128,1]
   per (graph, n-chunk), moving = one df column).
 - column-form outputs land as PSUM "colmats" [128, 128] which one PE
   transpose converts back to banded layout - no partition-scatter DMAs.
 - the MLP runs hid-partition: stationary [W1[:64]; b1; W1[64]] over a
   [66, NPC] moving operand (features + ones row + xs row); the xs row is
   refreshed each step by 8 small partition-gather DMAs from banded xs.
"""

import numpy as np
import ml_dtypes

import bass_rust
import concourse.bass as bass
import concourse.tile as tile
from concourse import mybir
from concourse.bass_utils import run_bass_kernel_spmd
from concourse.masks import make_identity

F32 = mybir.dt.float32
BF16 = mybir.dt.bfloat16
BF = ml_dtypes.bfloat16

B = 64          # graphs
NMAX = 2048     # nodes per graph (equal-size, sorted vals_batch)
F = 512         # projection basis dim
HID = 128
NFEAT = 64
NUM_STEPS = 8
STEP_ALPHA = 5.0
NCORES = 8
GPC = B // NCORES            # graphs per core = 8
NPC = GPC * NMAX             # nodes per core = 16384
NCH = NMAX // 128            # n-chunks per graph = 16
FCH = F // 128               # f-chunks = 4

AX = mybir.AxisListType
OP = mybir.AluOpType
ACT = mybir.ActivationFunctionType

_COMPILED = {}


def _split_sync_waits(nc, maxw=1):
    """Walrus in this container accepts at most one sync wait per
    instruction; split extra waits into preceding engine-local NoOps."""
    ctr = 0
    for f in nc.m.functions:
        for bb in f.blocks:
            insts = bb.instructions
            out = []
            changed = False
            for ins in insts:
                si = ins.sync_info
                waits = list(si.on_wait) if si is not None else []
                if len(waits) > maxw:
                    reg_waits = [w for w in waits if w.wait_reg is not None]
                    imm_waits = [w for w in waits if w.wait_reg is None]
                    nkeep = max(0, maxw - len(reg_waits))
                    keep = imm_waits[:nkeep]
                    extra = imm_waits[nkeep:]
                    for i in range(0, len(extra), maxw):
                        ctr += 1
                        nop = mybir.InstNoOp(name=f"wsplit-{ctr}", ins=[], outs=[])
                        nop.engine = ins.engine
                        nop.sync_info = bass_rust.SyncInfo(
                            on_wait=extra[i : i + maxw], on_update=[]
                        )
                        out.append(nop)
                    ins.sync_info = bass_rust.SyncInfo(
                        on_wait=reg_waits + keep, on_update=list(si.on_update)
                    )
                    changed = True
                out.append(ins)
            if changed:
                bb.instructions = out
    return ctr


def _tau_schedule():
    taus = []
    tau = 0.01
    for _ in range(NUM_STEPS):
        taus.append(tau)
        tau = max(tau * 0.5, 1e-5)
    return taus


def build_nc(debug=False, num_steps=NUM_STEPS, skip=(), reps=1):
    nc = bass.Bass()

    # ---------------- I/O ----------------
    P_d = nc.declare_dram_parameter("P", [128, GPC, NCH, F], BF16, isOutput=False)
    PT_d = nc.declare_dram_parameter("PT", [GPC, FCH, 128, NMAX], BF16, isOutput=False)
    nfTp_d = nc.declare_dram_parameter("nfTp", [NFEAT + 2, NPC], BF16, isOutput=False)
    xs0_d = nc.declare_dram_parameter("xs0", [128, 128], F32, isOutput=False)
    xsol_d = nc.declare_dram_parameter("xsol", [128, 128], F32, isOutput=False)
    w1a_d = nc.declare_dram_parameter("w1a", [NFEAT + 2, HID], BF16, isOutput=False)
    w2_d = nc.declare_dram_parameter("w2", [HID, 1], BF16, isOutput=False)
    b2_d = nc.declare_dram_parameter("b2", [1, 1], F32, isOutput=False)
    seg_d = nc.declare_dram_parameter("seg", [128, 128], F32, isOutput=False)
    seg8_d = nc.declare_dram_parameter("seg8", [GPC, 128], F32, isOutput=False)

    preds_o = nc.declare_dram_parameter("preds", [NUM_STEPS, NPC], F32, isOutput=True)
    labels_o = nc.declare_dram_parameter("labels", [NUM_STEPS, NPC], F32, isOutput=True)
    if debug:
        dbg_alpha = nc.declare_dram_parameter("dbg_alpha", [NUM_STEPS, GPC], F32, isOutput=True)
        dbg_df0 = nc.declare_dram_parameter("dbg_df0", [128, 32], F32, isOutput=True)
        dbg_y0 = nc.declare_dram_parameter("dbg_y0", [128, 128], F32, isOutput=True)
        dbg_d0 = nc.declare_dram_parameter("dbg_d0", [128, 128], F32, isOutput=True)

    taus = _tau_schedule()

    with tile.TileContext(nc) as tc:
        with (
            tc.tile_pool(name="res", bufs=1) as res,            # resident singles
            tc.tile_pool(name="ptp", bufs=7) as ptp,            # PT stream chunks
            tc.tile_pool(name="hp", bufs=3) as hp,              # relu'd hidden chunks
            tc.tile_pool(name="smt", bufs=1) as smt,            # small temps / state
            tc.tile_pool(name="mlp_ps", bufs=2, space="PSUM") as mlp_ps,
            tc.tile_pool(name="cm_ps", bufs=1, space="PSUM") as cm_ps,    # colmats (df/pred/y)
            tc.tile_pool(name="tr_ps", bufs=1, space="PSUM") as tr_ps,    # transposes
            tc.tile_pool(name="sg_ps", bufs=1, space="PSUM") as sg_ps,    # seg matmuls / micro
        ):
            # ---------------- constants / residents ----------------
            identf = res.tile([128, 128], F32, tag="identf")
            make_identity(nc, identf)
            identb = res.tile([128, 128], BF16, tag="identb")
            make_identity(nc, identb)
            onesf = res.tile([128, 1], F32, tag="onesf")
            nc.vector.memset(onesf, 1.0)
            onesb = res.tile([1, 128], BF16, tag="onesb")
            nc.vector.memset(onesb, 1.0)

            seg = res.tile([128, 128], F32, tag="seg")
            nc.sync.dma_start(out=seg, in_=seg_d[:])
            seg8 = res.tile([GPC, 128], F32, tag="seg8")
            nc.sync.dma_start(out=seg8, in_=seg8_d[:])

            w1a = res.tile([NFEAT + 2, HID], BF16, tag="w1a")
            nc.sync.dma_start(out=w1a, in_=w1a_d[:])
            w2 = res.tile([HID, 1], BF16, tag="w2")
            nc.sync.dma_start(out=w2, in_=w2_d[:])
            b2c = res.tile([128, 1], F32, tag="b2c")
            nc.sync.dma_start(
                out=b2c,
                in_=bass.AP(tensor=b2_d, offset=0, ap=[[0, 128], [1, 1]]),
            )

            # state (small; load before the big residents so step 0 can start)
            xs = res.tile([128, 128], F32, tag="xs")
            nc.sync.dma_start(out=xs, in_=xs0_d[:])
            xsol = res.tile([128, 128], F32, tag="xsol")
            nc.sync.dma_start(out=xsol, in_=xsol_d[:])

            # MLP moving operand: rows 0..63 features, row 64 ones (folds b1
            # via the extra row of w1a), row 65 = xs (refreshed per step);
            # split the load so MLP chunks can start while later pieces
            # stream.
            nfTp = res.tile([NFEAT + 2, NPC], BF16, tag="nfTp")
            for q in range(4):
                nc.sync.dma_start(
                    out=nfTp[:, 4096 * q : 4096 * (q + 1)],
                    in_=nfTp_d[:, 4096 * q : 4096 * (q + 1)],
                )

            # big resident P (bf16); split by graph across DMAs
            sbP = res.tile([128, GPC, NCH, F], BF16, tag="sbP")
            for g in range(GPC):
                nc.scalar.dma_start(out=sbP[:, g], in_=P_d[:, g])

            for rep in range(reps):
              if rep > 0:
                # re-run the whole workload on the same inputs (throughput
                # measurement); reset the solver state
                nc.sync.dma_start(out=xs, in_=xs0_d[:])
              for s in range(num_steps):
                tau = taus[s]

                # ---- A: MLP  h = relu(W1f^T nf + b1 + W1x^T xs) ----
                xs_bf = smt.tile([128, 128], BF16, tag="xs_bf")
                nc.vector.tensor_copy(xs_bf, xs)
                # refresh the xs row of the moving operand (8 small
                # partition-gather DMAs; issued from gpsimd = cheap)
                for g in range(GPC):
                    nc.gpsimd.dma_start(
                        out=nfTp[
                            NFEAT + 1 : NFEAT + 2, 2048 * g : 2048 * (g + 1)
                        ].rearrange("o (p c) -> o p c", p=16),
                        in_=xs_bf[16 * g : 16 * g + 16, :],
                    )
                # recv = 1/(xs+tau): independent of pred, compute early
                recv = smt.tile([128, 128], F32, tag="recv")
                nc.vector.tensor_scalar_add(recv, xs, float(tau))
                nc.vector.reciprocal(recv, recv)
                nc.vector.tensor_scalar(
                    out=recv, in0=recv, scalar1=float(3.0 * tau), scalar2=None,
                    op0=OP.mult,
                )

                if "dchain" not in skip:
                    diff = smt.tile([128, 128], F32, tag="diff")
                    nc.vector.tensor_sub(diff, xsol, xs)
                    adiff = smt.tile([128, 128], F32, tag="adiff", bufs=1)
                    nc.vector.scalar_tensor_tensor(
                        out=adiff, in0=diff, scalar=-1.0, in1=diff,
                        op0=OP.mult, op1=OP.max,
                    )
                    lab_part = smt.tile([128, 1], F32, tag="lab_part")
                    nc.vector.tensor_reduce(
                        out=lab_part, in_=adiff, axis=AX.X, op=OP.add
                    )
                    ls_ps = sg_ps.tile([128, 1], F32, tag="sg")
                    nc.tensor.matmul(ls_ps, seg, lab_part, start=True, stop=True)
                    lscale = smt.tile([128, 1], F32, tag="lscale")
                    nc.vector.tensor_scalar_max(lscale, ls_ps, 1e-8)
                    nc.vector.reciprocal(lscale, lscale)
                    label = smt.tile([128, 128], F32, tag="label")
                    nc.vector.tensor_scalar(
                        out=label, in0=diff, scalar1=lscale, scalar2=None, op0=OP.mult
                    )
                    nc.sync.dma_start(
                        out=labels_o[s].rearrange("(p c) -> p c", p=128), in_=label
                    )

                pred_ps = cm_ps.tile([128, 128], F32, tag="pred_ps")
                NWM = 64 if "mlpmm" in skip else 512
                for j in range(32):
                    hpsum = mlp_ps.tile([128, 512], F32, tag="hpsum")
                    nc.tensor.matmul(
                        hpsum[:, 0:NWM],
                        w1a,
                        nfTp[:, 512 * j : 512 * j + NWM],
                        start=True,
                        stop=True,
                    )
                    hpos = hp.tile([128, 512], BF16, tag="hpos")
                    NWR = 64 if "relu" in skip else 512
                    if j % 2 == 0:
                        nc.scalar.activation(
                            out=hpos[:, 0:NWR], in_=hpsum[:, 0:NWR], func=ACT.Relu
                        )
                    else:
                        nc.vector.tensor_scalar(
                            out=hpos[:, 0:NWR], in0=hpsum[:, 0:NWR],
                            scalar1=0.0, scalar2=None, op0=OP.max,
                        )
                    if NWR < 512:
                        nc.vector.tensor_copy(hpos[:, NWR:512], hpos[:, 0:512 - NWR])
                    # W2 column-form: one pred column per 128-node chunk
                    for t in range(4):
                        nc.tensor.matmul(
                            pred_ps[:, 4 * j + t : 4 * j + t + 1],
                            hpos[:, 128 * t : 128 * (t + 1)],
                            w2,
                            start=True,
                            stop=True,
                        )
                # colmat -> banded: evac (+b2) then one PE transpose
                pred_cm = smt.tile([128, 128], BF16, tag="pred_cm")
                nc.vector.tensor_scalar(
                    out=pred_cm, in0=pred_ps, scalar1=b2c, scalar2=None, op0=OP.add
                )
                predT_ps = tr_ps.tile([128, 128], BF16, tag="tr", name="predT_ps")
                nc.tensor.transpose(predT_ps, pred_cm, identb)
                pred = smt.tile([128, 128], BF16, tag="pred", bufs=2)
                nc.vector.tensor_copy(pred, predT_ps)

                # ---- B: l1norm scales + labels + direction d ----
                if "dchain" in skip:
                    diff0 = smt.tile([128, 128], F32, tag="diff", bufs=1)
                    nc.vector.tensor_sub(diff0, xsol, xs)
                    nc.sync.dma_start(
                        out=labels_o[s].rearrange("(p c) -> p c", p=128), in_=diff0
                    )
                    nc.gpsimd.dma_start(
                        out=preds_o[s].rearrange("(p c) -> p c", p=128), in_=pred
                    )
                    d_c = smt.tile([128, 128], BF16, tag="d_c")
                    nc.vector.memset(d_c, 0.01)
                else:
                    # scale-folded direction: d' = pred + s * 3tau/(xs+tau)
                    # (s = sum_g |pred|; the true d = d'/s, and the division
                    # is deferred into the line-search alpha, which is
                    # scale-invariant up to the min-with-STEP_ALPHA).
                    # s chain runs on the colmat, in parallel with the pred
                    # transpose.
                    apred = smt.tile([128, 128], F32, tag="apred", bufs=1)
                    nc.vector.scalar_tensor_tensor(
                        out=apred, in0=pred_cm, scalar=-1.0, in1=pred_cm,
                        op0=OP.mult, op1=OP.max,
                    )
                    part8 = smt.tile([128, GPC], F32, tag="part8")
                    nc.vector.tensor_reduce(
                        out=part8,
                        in_=apred.rearrange("p (g c) -> p g c", g=GPC),
                        axis=AX.X,
                        op=OP.add,
                    )
                    s8c_ps = sg_ps.tile([128, 1], F32, tag="sg")
                    nc.tensor.matmul(
                        s8c_ps[0:GPC, :], part8, onesf, start=True, stop=True
                    )
                    s8c = smt.tile([GPC, 1], F32, tag="s8c")
                    nc.vector.tensor_scalar_max(s8c, s8c_ps[0:GPC, :], 1e-8)
                    scol_ps = sg_ps.tile([128, 1], F32, tag="sg")
                    nc.tensor.matmul(scol_ps, seg8, s8c, start=True, stop=True)
                    s_col = smt.tile([128, 1], F32, tag="s_col")
                    nc.vector.tensor_copy(s_col, scol_ps)
                    # s as a row for the alpha stage (off the d critical path)
                    s8r_ps = sg_ps.tile([1, 128], F32, tag="sgw", name="s8r_ps")
                    nc.tensor.transpose(s8r_ps[:, 0:GPC], s8c, identf[0:GPC, 0:GPC])
                    s8row = smt.tile([1, GPC], F32, tag="s8row")
                    nc.vector.tensor_copy(s8row, s8r_ps[:, 0:GPC])

                    nc.gpsimd.dma_start(
                        out=preds_o[s].rearrange("(p c) -> p c", p=128), in_=pred
                    )
                    d_bf = smt.tile([128, 128], BF16, tag="d_bf")
                    nc.vector.scalar_tensor_tensor(
                        out=d_bf, in0=recv, scalar=s_col, in1=pred,
                        op0=OP.mult, op1=OP.add,
                    )
                    if debug and s == 0:
                        d_dbg = smt.tile([128, 128], F32, tag="d_dbg")
                        nc.vector.tensor_copy(d_dbg, d_bf)
                        nc.sync.dma_start(out=dbg_d0[:], in_=d_dbg)
                    dct_ps = tr_ps.tile([128, 128], BF16, tag="tr", name="dct_ps")
                    nc.tensor.transpose(dct_ps, d_bf, identb)
                    d_c = smt.tile([128, 128], BF16, tag="d_c")
                    nc.vector.tensor_copy(d_c, dct_ps)

                # ---- C: einsum1 (column form)  df[g,f] = sum_n P[g,n,f] d[g,n]
                df_ps = cm_ps.tile([128, 32], F32, tag="df_ps")
                NK1 = 2 if "e1mm" in skip else NCH
                dfb67 = smt.tile([128, 8], BF16, tag="dfb67")
                for g in [6, 7, 0, 1, 2, 3, 4, 5]:
                    for j in range(FCH):
                        for k in range(NK1):
                            nc.tensor.matmul(
                                df_ps[:, 4 * g + j : 4 * g + j + 1],
                                sbP[:, g, k, 128 * j : 128 * (j + 1)],
                                d_c[:, 16 * g + k : 16 * g + k + 1],
                                start=(k == 0),
                                stop=(k == NK1 - 1),
                            )
                    if g == 7:
                        # early evac of the DVE-path df columns so that path
                        # overlaps the remaining PE einsum work
                        nc.vector.tensor_copy(dfb67, df_ps[:, 24:32])
                dfb = smt.tile([128, 32], BF16, tag="dfb")
                nc.vector.tensor_copy(dfb, df_ps)
                if debug and s == 0:
                    dfb32 = smt.tile([128, 32], F32, tag="dfb32")
                    nc.vector.tensor_copy(dfb32, df_ps)
                    nc.sync.dma_start(out=dbg_df0[:], in_=dfb32)

                # ---- D: einsum2 (column form)  y[g,n] = sum_f PT[g,f,n] df[g,f]
                y_ps = cm_ps.tile([128, 128], F32, tag="y_ps")
                NK2 = 2 if "e2mm" in skip else NCH
                NGPE = GPC - 2   # last 2 graphs run einsum2 on DVE from resident P
                for g in range(NGPE):
                    ptts = []
                    for j in range(FCH):
                        ptt = ptp.tile([128, NMAX], BF16, tag="ptt", name="ptt")
                        if "ptdma" not in skip:
                            nc.sync.dma_start(out=ptt, in_=PT_d[g, j])
                        else:
                            nc.sync.dma_start(out=ptt[:, 0:256], in_=PT_d[g, j, :, 0:256])
                        ptts.append(ptt)
                    for k in range(NK2):
                        for j in range(FCH):
                            nc.tensor.matmul(
                                y_ps[:, 16 * g + k : 16 * g + k + 1],
                                ptts[j][:, 128 * k : 128 * (k + 1)],
                                dfb[:, 4 * g + j : 4 * g + j + 1],
                                start=(j == 0),
                                stop=(j == FCH - 1),
                            )
                # DVE path: y'[g,128k+r] = sum_f P[g,128k+r,f] * df'[g,f]
                y_dve = smt.tile([128, 2 * NCH], F32, tag="y_dve")
                for gi in range(2):
                    g = NGPE + gi
                    # df columns -> one row at partition 0 -> broadcast tile
                    df_row = smt.tile([1, F], BF16, tag="df_row", bufs=2)
                    for j in range(FCH):
                        dfr_ps = tr_ps.tile(
                            [128, 128], BF16, tag="tr", name=f"dfr{g}_{j}"
                        )
                        nc.tensor.transpose(
                            dfr_ps[0:1, :],
                            dfb67[:, 4 * gi + j : 4 * gi + j + 1],
                            identb,
                        )
                        nc.vector.tensor_copy(
                            df_row[:, 128 * j : 128 * (j + 1)], dfr_ps[0:1, :]
                        )
                    bc_ps = mlp_ps.tile([128, 512], F32, tag="hpsum", name=f"bcps{g}")
                    nc.tensor.matmul(bc_ps, onesb, df_row, start=True, stop=True)
                    df_bc = smt.tile([128, F], BF16, tag="df_bc", bufs=1)
                    nc.vector.tensor_copy(df_bc, bc_ps)
                    junk = smt.tile([128, F], BF16, tag="e2junk", bufs=1)
                    for k in range(NK2):
                        nc.vector.tensor_mul(junk, sbP[:, g, k, :], df_bc)
                        nc.vector.tensor_reduce(
                            out=y_dve[:, 16 * gi + k : 16 * gi + k + 1],
                            in_=junk, axis=AX.X, op=OP.add,
                        )
                y_cm = smt.tile([128, 128], BF16, tag="y_cm")
                nc.vector.tensor_copy(y_cm[:, 0 : 16 * NGPE], y_ps[:, 0 : 16 * NGPE])
                nc.vector.tensor_copy(y_cm[:, 16 * NGPE : 128], y_dve)
                yT_ps = tr_ps.tile([128, 128], BF16, tag="tr", name="yT_ps")
                nc.tensor.transpose(yT_ps, y_cm, identb)
                y_pm = smt.tile([128, 128], BF16, tag="y_pm", bufs=2)
                nc.vector.tensor_copy(y_pm, yT_ps)
                if debug and s == 0:
                    y_dbg = smt.tile([128, 128], F32, tag="y_dbg")
                    nc.vector.tensor_copy(y_dbg, y_pm)
                    nc.sync.dma_start(out=dbg_y0[:], in_=y_dbg)

                # ---- E: line search + state update ----
                if "alpha" in skip:
                    acol0 = smt.tile([128, 1], F32, tag="acol")
                    nc.vector.memset(acol0, 0.05)
                    ay0 = smt.tile([128, 128], F32, tag="ay", bufs=1)
                    nc.vector.tensor_scalar(
                        out=ay0, in0=y_pm, scalar1=acol0, scalar2=None, op0=OP.mult
                    )
                    nc.vector.tensor_add(xs, xs, ay0)
                    continue
                q = smt.tile([128, 128], F32, tag="q", bufs=1)
                nc.vector.tensor_scalar(
                    out=q, in0=y_pm, scalar1=-1.0, scalar2=1e-30,
                    op0=OP.mult, op1=OP.max,
                )
                nc.vector.reciprocal(q, q)
                stp = smt.tile([128, 128], F32, tag="stp", bufs=1)
                nc.vector.tensor_mul(stp, xs, q)
                smin = smt.tile([128, 1], F32, tag="smin")
                nc.vector.tensor_reduce(out=smin, in_=stp, axis=AX.X, op=OP.min)
                # per-graph min: transpose -> row -> seg-min -> back to column
                smin_ps = sg_ps.tile([1, 128], F32, tag="sgw")
                nc.tensor.transpose(smin_ps, smin, identf)
                smin_row = smt.tile([1, 128], F32, tag="smin_row")
                nc.vector.tensor_copy(smin_row, smin_ps)
                amin_row = smt.tile([1, GPC], F32, tag="amin_row")
                nc.vector.tensor_reduce(
                    out=amin_row,
                    in_=smin_row.rearrange("o (g b) -> o g b", g=GPC),
                    axis=AX.X,
                    op=OP.min,
                )
                if "dchain" not in skip:
                    nc.vector.tensor_mul(amin_row, amin_row, s8row)
                nc.vector.tensor_scalar(
                    out=amin_row, in0=amin_row, scalar1=float(STEP_ALPHA),
                    scalar2=0.995, op0=OP.min, op1=OP.mult,
                )
                if "dchain" not in skip:
                    s8inv = smt.tile([1, GPC], F32, tag="s8inv")
                    nc.vector.reciprocal(s8inv, s8row)
                    nc.vector.tensor_mul(amin_row, amin_row, s8inv)
                if debug:
                    nc.sync.dma_start(out=dbg_alpha[s], in_=amin_row)
                a8_ps = sg_ps.tile([GPC, 1], F32, tag="sg")
                nc.tensor.transpose(a8_ps, amin_row, identf[0:1, 0:1])
                a8 = smt.tile([GPC, 1], F32, tag="a8")
                nc.vector.tensor_copy(a8, a8_ps)
                acol_ps = sg_ps.tile([128, 1], F32, tag="sg")
                nc.tensor.matmul(acol_ps, seg8, a8, start=True, stop=True)
                acol = smt.tile([128, 1], F32, tag="acol")
                nc.vector.tensor_copy(acol, acol_ps)

                ay = smt.tile([128, 128], F32, tag="ay", bufs=1)
                nc.vector.tensor_scalar(
                    out=ay, in0=y_pm, scalar1=acol, scalar2=None, op0=OP.mult
                )
                nc.vector.tensor_add(xs, xs, ay)

    _split_sync_waits(nc, maxw=1)
    return nc


def _seg_mats():
    seg = np.zeros((128, 128), np.float32)
    for g in range(GPC):
        seg[16 * g : 16 * g + 16, 16 * g : 16 * g + 16] = 1.0
    seg8 = np.zeros((GPC, 128), np.float32)
    for g in range(GPC):
        seg8[g, 16 * g : 16 * g + 16] = 1.0
    return seg, seg8


def _prep_core_inputs(core, proj, x_start, x_solution, node_feat, W1, b1, W2, b2):
    g0 = core * GPC
    n0 = core * NPC
    Pc = proj[g0 : g0 + GPC]  # [8, 2048, 512] f32
    P_bf = np.ascontiguousarray(
        Pc.reshape(GPC, NCH, 128, F).transpose(2, 0, 1, 3)
    ).astype(BF)
    PT_bf = np.ascontiguousarray(Pc.transpose(0, 2, 1)).reshape(
        GPC, FCH, 128, NMAX
    ).astype(BF)
    nfTp = np.empty((NFEAT + 2, NPC), BF)
    nfTp[0:NFEAT] = np.ascontiguousarray(node_feat[n0 : n0 + NPC].T).astype(BF)
    nfTp[NFEAT] = np.ones((NPC,), BF)
    nfTp[NFEAT + 1] = np.zeros((NPC,), BF)  # xs row, written on device
    w1a = np.empty((NFEAT + 2, HID), BF)
    w1a[0:NFEAT] = W1[0:NFEAT].astype(BF)
    w1a[NFEAT] = b1.astype(BF)
    w1a[NFEAT + 1] = W1[NFEAT].astype(BF)
    seg, seg8 = _seg_mats()
    return {
        "P": P_bf,
        "PT": PT_bf,
        "nfTp": nfTp,
        "xs0": x_start[n0 : n0 + NPC].reshape(128, 128).astype(np.float32),
        "xsol": x_solution[n0 : n0 + NPC].reshape(128, 128).astype(np.float32),
        "w1a": w1a,
        "w2": W2.reshape(HID, 1).astype(BF),
        "b2": b2.reshape(1, 1).astype(np.float32),
        "seg": seg,
        "seg8": seg8,
    }


def _numpy_fallback(x_start, x_solution, node_feat, proj_matrix, W1, b1, W2, b2, batch):
    """General (ragged) reference implementation in numpy, used only if
    vals_batch is not the expected equal-size pattern."""
    nb = proj_matrix.shape[0]
    batch = batch.astype(np.int64)
    counts = np.bincount(batch, minlength=nb)
    offsets = np.cumsum(counts) - counts
    pos = np.arange(batch.shape[0]) - offsets[batch]

    def l1norm(x):
        s = np.zeros(nb, x.dtype)
        np.add.at(s, batch, np.abs(x))
        return x / np.clip(s, 1e-8, None)[batch]

    def to_dense(x):
        dense = np.zeros((nb, NMAX), x.dtype)
        m = pos < NMAX
        dense[batch[m], pos[m]] = x[m]
        return dense

    def line_search(x, dvec):
        neg = dvec < 0
        step = np.where(neg, x / np.where(neg, -dvec, 1.0), STEP_ALPHA)
        a = np.full(nb, np.inf, step.dtype)
        np.minimum.at(a, batch, step)
        return np.minimum(a, STEP_ALPHA)[batch]

    def gnn(x):
        h = np.concatenate([node_feat, x[:, None]], axis=-1)
        h = np.maximum(h @ W1 + b1, 0.0)
        return (h @ W2 + b2)[:, 0]

    tau = 0.01
    xs = x_start.astype(np.float32)
    preds, labels = [], []
    for _ in range(NUM_STEPS):
        pred = gnn(xs)
        preds.append(pred)
        labels.append(l1norm(x_solution - xs))
        p = l1norm(pred)
        direction = p + 3.0 * tau / (xs + tau)
        tau = max(tau * 0.5, 1e-5)
        d_dense = to_dense(direction)
        df = np.einsum("bnf,bn->bf", proj_matrix, d_dense)
        proj_dense = np.einsum("bnf,bf->bn", proj_matrix, df)
        proj_flat = proj_dense[batch, np.minimum(pos, NMAX - 1)]
        proj_flat = np.where(pos < NMAX, proj_flat, 0.0)
        alpha = line_search(xs, proj_flat) * 0.995
        xs = xs + alpha * proj_flat
    return np.stack(preds, 1).astype(np.float32), np.stack(labels, 1).astype(np.float32)


def run_on_hw(inputs_list, debug=False):
    key = "dbg" if debug else "plain"
    if key not in _COMPILED:
        _COMPILED[key] = build_nc(debug=debug)
    nc = _COMPILED[key]
    return run_bass_kernel_spmd(nc, inputs_list, list(range(NCORES))).results


def kernel(x_start, x_solution, node_feat, proj_matrix, W1, b1, W2, b2, vals_batch):
    expected = np.repeat(np.arange(B, dtype=np.int64), NMAX)
    vb = np.asarray(vals_batch)
    if vb.shape != expected.shape or not np.array_equal(
        vb.astype(np.int64), expected
    ):
        return _numpy_fallback(
            np.asarray(x_start, np.float32),
            np.asarray(x_solution, np.float32),
            np.asarray(node_feat, np.float32),
            np.asarray(proj_matrix, np.float32),
            np.asarray(W1, np.float32),
            np.asarray(b1, np.float32),
            np.asarray(W2, np.float32),
            np.asarray(b2, np.float32),
            vb,
        )

    x_start = np.asarray(x_start, np.float32)
    x_solution = np.asarray(x_solution, np.float32)
    node_feat = np.asarray(node_feat, np.float32)
    proj_matrix = np.asarray(proj_matrix, np.float32)
    W1 = np.asarray(W1, np.float32)
    b1 = np.asarray(b1, np.float32)
    W2 = np.asarray(W2, np.float32)
    b2 = np.asarray(b2, np.float32)

    ins = [
        _prep_core_inputs(c, proj_matrix, x_start, x_solution, node_feat, W1, b1, W2, b2)
        for c in range(NCORES)
    ]
    results = run_on_hw(ins)
    preds = np.concatenate(
        [results[c]["preds"].T for c in range(NCORES)], axis=0
    ).astype(np.float32)
    labels = np.concatenate(
        [results[c]["labels"].T for c in range(NCORES)], axis=0
    ).astype(np.float32)
    return preds, labels


# revision 25
# speedup vs baseline: 1.0933x; 1.0859x over previous
"""Trainium2 Bass kernel for nn_CycleGNN (8-step projected-direction solver).

Contract: kernel(**inputs) takes the FULL unsharded numpy inputs (keyed as in
setup_inputs()) and returns the full output (preds, labels), each
[131072, 8] float32.  Internally shards the 64 graphs across 8 NeuronCores
(8 graphs per core, graphs never interact -> no collectives), runs a Tile
kernel via run_bass_kernel_spmd, and re-assembles on the host.

Device-side layout notes (per core, 8 graphs, 16384 nodes):
 - per-node state is "p-major banded" [128, 128]: tile[p, c] = v[p*128 + c];
   graph g owns partitions [16g, 16g+16).
 - P (bf16) stays resident in SBUF ([128, 8, 16, 512]) and is the stationary
   operand of column-form einsum1 matmuls (out df column [128,1] per
   (graph, f-chunk), moving operand = one d column).
 - PT (bf16) streams from DRAM each step ([128, 2048] per (g, f-chunk)) and
   is the stationary operand of column-form einsum2 (out y column [128,1]
   per (graph, n-chunk), moving = one df column).
 - column-form outputs land as PSUM "colmats" [128, 128] which one PE
   transpose converts back to banded layout - no partition-scatter DMAs.
 - the MLP runs hid-partition: stationary [W1[:64]; b1; W1[64]] over a
   [66, NPC] moving operand (features + ones row + xs row); the xs row is
   refreshed each step by 8 small partition-gather DMAs from banded xs.
"""

import numpy as np
import ml_dtypes

import bass_rust
import concourse.bass as bass
import concourse.tile as tile
from concourse import mybir
from concourse.bass_utils import run_bass_kernel_spmd
from concourse.masks import make_identity

F32 = mybir.dt.float32
BF16 = mybir.dt.bfloat16
BF = ml_dtypes.bfloat16

B = 64          # graphs
NMAX = 2048     # nodes per graph (equal-size, sorted vals_batch)
F = 512         # projection basis dim
HID = 128
NFEAT = 64
NUM_STEPS = 8
STEP_ALPHA = 5.0
NCORES = 8
GPC = B // NCORES            # graphs per core = 8
NPC = GPC * NMAX             # nodes per core = 16384
NCH = NMAX // 128            # n-chunks per graph = 16
FCH = F // 128               # f-chunks = 4

AX = mybir.AxisListType
OP = mybir.AluOpType
ACT = mybir.ActivationFunctionType

_COMPILED = {}


def _split_sync_waits(nc, maxw=1):
    """Walrus in this container accepts at most one sync wait per
    instruction; split extra waits into preceding engine-local NoOps."""
    ctr = 0
    for f in nc.m.functions:
        for bb in f.blocks:
            insts = bb.instructions
            out = []
            changed = False
            for ins in insts:
                si = ins.sync_info
                waits = list(si.on_wait) if si is not None else []
                if len(waits) > maxw:
                    reg_waits = [w for w in waits if w.wait_reg is not None]
                    imm_waits = [w for w in waits if w.wait_reg is None]
                    nkeep = max(0, maxw - len(reg_waits))
                    keep = imm_waits[:nkeep]
                    extra = imm_waits[nkeep:]
                    for i in range(0, len(extra), maxw):
                        ctr += 1
                        nop = mybir.InstNoOp(name=f"wsplit-{ctr}", ins=[], outs=[])
                        nop.engine = ins.engine
                        nop.sync_info = bass_rust.SyncInfo(
                            on_wait=extra[i : i + maxw], on_update=[]
                        )
                        out.append(nop)
                    ins.sync_info = bass_rust.SyncInfo(
                        on_wait=reg_waits + keep, on_update=list(si.on_update)
                    )
                    changed = True
                out.append(ins)
            if changed:
                bb.instructions = out
    return ctr


def _tau_schedule():
    taus = []
    tau = 0.01
    for _ in range(NUM_STEPS):
        taus.append(tau)
        tau = max(tau * 0.5, 1e-5)
    return taus


def build_nc(debug=False, num_steps=NUM_STEPS, skip=(), reps=1):
    nc = bass.Bass()

    # ---------------- I/O ----------------
    P_d = nc.declare_dram_parameter("P", [128, GPC, NCH, F], BF16, isOutput=False)
    PT_d = nc.declare_dram_parameter("PT", [GPC, FCH, 128, NMAX], BF16, isOutput=False)
    nfTp_d = nc.declare_dram_parameter("nfTp", [NFEAT + 2, NPC], BF16, isOutput=False)
    xs0_d = nc.declare_dram_parameter("xs0", [128, 128], F32, isOutput=False)
    xsol_d = nc.declare_dram_parameter("xsol", [128, 128], F32, isOutput=False)
    w1a_d = nc.declare_dram_parameter("w1a", [NFEAT + 2, HID], BF16, isOutput=False)
    w2_d = nc.declare_dram_parameter("w2", [HID, 1], BF16, isOutput=False)
    b2_d = nc.declare_dram_parameter("b2", [1, 1], F32, isOutput=False)
    seg_d = nc.declare_dram_parameter("seg", [128, 128], F32, isOutput=False)
    seg8_d = nc.declare_dram_parameter("seg8", [GPC, 128], F32, isOutput=False)

    preds_o = nc.declare_dram_parameter("preds", [NUM_STEPS, NPC], F32, isOutput=True)
    labels_o = nc.declare_dram_parameter("labels", [NUM_STEPS, NPC], F32, isOutput=True)
    if debug:
        dbg_alpha = nc.declare_dram_parameter("dbg_alpha", [NUM_STEPS, GPC], F32, isOutput=True)
        dbg_df0 = nc.declare_dram_parameter("dbg_df0", [128, 32], F32, isOutput=True)
        dbg_y0 = nc.declare_dram_parameter("dbg_y0", [128, 128], F32, isOutput=True)
        dbg_d0 = nc.declare_dram_parameter("dbg_d0", [128, 128], F32, isOutput=True)

    taus = _tau_schedule()

    with tile.TileContext(nc) as tc:
        with (
            tc.tile_pool(name="res", bufs=1) as res,            # resident singles
            tc.tile_pool(name="ptp", bufs=7) as ptp,            # PT stream chunks
            tc.tile_pool(name="hp", bufs=3) as hp,              # relu'd hidden chunks
            tc.tile_pool(name="smt", bufs=1) as smt,            # small temps / state
            tc.tile_pool(name="mlp_ps", bufs=2, space="PSUM") as mlp_ps,
            tc.tile_pool(name="cm_ps", bufs=1, space="PSUM") as cm_ps,    # colmats (df/pred/y)
            tc.tile_pool(name="tr_ps", bufs=1, space="PSUM") as tr_ps,    # transposes
            tc.tile_pool(name="sg_ps", bufs=1, space="PSUM") as sg_ps,    # seg matmuls / micro
        ):
            # ---------------- constants / residents ----------------
            identf = res.tile([128, 128], F32, tag="identf")
            make_identity(nc, identf)
            identb = res.tile([128, 128], BF16, tag="identb")
            make_identity(nc, identb)
            onesf = res.tile([128, 1], F32, tag="onesf")
            nc.vector.memset(onesf, 1.0)
            onesb = res.tile([1, 128], BF16, tag="onesb")
            nc.vector.memset(onesb, 1.0)

            seg = res.tile([128, 128], F32, tag="seg")
            nc.sync.dma_start(out=seg, in_=seg_d[:])
            seg8 = res.tile([GPC, 128], F32, tag="seg8")
            nc.sync.dma_start(out=seg8, in_=seg8_d[:])

            w1a = res.tile([NFEAT + 2, HID], BF16, tag="w1a")
            nc.sync.dma_start(out=w1a, in_=w1a_d[:])
            w2 = res.tile([HID, 1], BF16, tag="w2")
            nc.sync.dma_start(out=w2, in_=w2_d[:])
            b2c = res.tile([128, 1], F32, tag="b2c")
            nc.sync.dma_start(
                out=b2c,
                in_=bass.AP(tensor=b2_d, offset=0, ap=[[0, 128], [1, 1]]),
            )

            # state (small; load before the big residents so step 0 can start)
            xs = res.tile([128, 128], F32, tag="xs")
            nc.sync.dma_start(out=xs, in_=xs0_d[:])
            xsol = res.tile([128, 128], F32, tag="xsol")
            nc.sync.dma_start(out=xsol, in_=xsol_d[:])

            # MLP moving operand: rows 0..63 features, row 64 ones (folds b1
            # via the extra row of w1a), row 65 = xs (refreshed per step);
            # split the load so MLP chunks can start while later pieces
            # stream.
            nfTp = res.tile([NFEAT + 2, NPC], BF16, tag="nfTp")
            for q in range(4):
                nc.sync.dma_start(
                    out=nfTp[:, 4096 * q : 4096 * (q + 1)],
                    in_=nfTp_d[:, 4096 * q : 4096 * (q + 1)],
                )

            # big resident P (bf16); split by graph across DMAs
            sbP = res.tile([128, GPC, NCH, F], BF16, tag="sbP")
            for g in range(GPC):
                nc.scalar.dma_start(out=sbP[:, g], in_=P_d[:, g])

            for rep in range(reps):
              if rep > 0:
                # re-run the whole workload on the same inputs (throughput
                # measurement); reset the solver state
                nc.sync.dma_start(out=xs, in_=xs0_d[:])
              for s in range(num_steps):
                tau = taus[s]

                # ---- A: MLP  h = relu(W1f^T nf + b1 + W1x^T xs) ----
                xs_bf = smt.tile([128, 128], BF16, tag="xs_bf")
                nc.vector.tensor_copy(xs_bf, xs)
                # refresh the xs row of the moving operand (8 small
                # partition-gather DMAs; issued from gpsimd = cheap)
                for g in range(GPC):
                    nc.gpsimd.dma_start(
                        out=nfTp[
                            NFEAT + 1 : NFEAT + 2, 2048 * g : 2048 * (g + 1)
                        ].rearrange("o (p c) -> o p c", p=16),
                        in_=xs_bf[16 * g : 16 * g + 16, :],
                    )
                last = s == num_steps - 1
                # recv = 3tau/(xs+tau): independent of pred, compute early
                # (only feeds the direction, which the last step never uses)
                if not last:
                    recv = smt.tile([128, 128], F32, tag="recv")
                    nc.vector.tensor_scalar_add(recv, xs, float(tau))
                    nc.vector.reciprocal(recv, recv)
                    nc.vector.tensor_scalar(
                        out=recv, in0=recv, scalar1=float(3.0 * tau), scalar2=None,
                        op0=OP.mult,
                    )

                if "dchain" not in skip:
                    diff = smt.tile([128, 128], F32, tag="diff")
                    nc.vector.tensor_sub(diff, xsol, xs)
                    adiff = smt.tile([128, 128], F32, tag="adiff", bufs=1)
                    nc.vector.scalar_tensor_tensor(
                        out=adiff, in0=diff, scalar=-1.0, in1=diff,
                        op0=OP.mult, op1=OP.max,
                    )
                    lab_part = smt.tile([128, 1], F32, tag="lab_part")
                    nc.vector.tensor_reduce(
                        out=lab_part, in_=adiff, axis=AX.X, op=OP.add
                    )
                    ls_ps = sg_ps.tile([128, 1], F32, tag="sg")
                    nc.tensor.matmul(ls_ps, seg, lab_part, start=True, stop=True)
                    lscale = smt.tile([128, 1], F32, tag="lscale")
                    nc.vector.tensor_scalar_max(lscale, ls_ps, 1e-8)
                    nc.vector.reciprocal(lscale, lscale)
                    label = smt.tile([128, 128], F32, tag="label")
                    nc.vector.tensor_scalar(
                        out=label, in0=diff, scalar1=lscale, scalar2=None, op0=OP.mult
                    )
                    nc.sync.dma_start(
                        out=labels_o[s].rearrange("(p c) -> p c", p=128), in_=label
                    )

                pred_ps = cm_ps.tile([128, 128], F32, tag="pred_ps")
                NWM = 64 if "mlpmm" in skip else 512
                for j in range(32):
                    hpsum = mlp_ps.tile([128, 512], F32, tag="hpsum")
                    nc.tensor.matmul(
                        hpsum[:, 0:NWM],
                        w1a,
                        nfTp[:, 512 * j : 512 * j + NWM],
                        start=True,
                        stop=True,
                    )
                    hpos = hp.tile([128, 512], BF16, tag="hpos")
                    NWR = 64 if "relu" in skip else 512
                    if j % 2 == 0:
                        nc.scalar.activation(
                            out=hpos[:, 0:NWR], in_=hpsum[:, 0:NWR], func=ACT.Relu
                        )
                    else:
                        nc.vector.tensor_scalar(
                            out=hpos[:, 0:NWR], in0=hpsum[:, 0:NWR],
                            scalar1=0.0, scalar2=None, op0=OP.max,
                        )
                    if NWR < 512:
                        nc.vector.tensor_copy(hpos[:, NWR:512], hpos[:, 0:512 - NWR])
                    # W2 column-form: one pred column per 128-node chunk
                    for t in range(4):
                        nc.tensor.matmul(
                            pred_ps[:, 4 * j + t : 4 * j + t + 1],
                            hpos[:, 128 * t : 128 * (t + 1)],
                            w2,
                            start=True,
                            stop=True,
                        )
                # colmat -> banded: evac (+b2) then one PE transpose
                pred_cm = smt.tile([128, 128], BF16, tag="pred_cm")
                nc.vector.tensor_scalar(
                    out=pred_cm, in0=pred_ps, scalar1=b2c, scalar2=None, op0=OP.add
                )
                predT_ps = tr_ps.tile([128, 128], BF16, tag="tr", name="predT_ps")
                nc.tensor.transpose(predT_ps, pred_cm, identb)
                pred = smt.tile([128, 128], BF16, tag="pred", bufs=2)
                nc.vector.tensor_copy(pred, predT_ps)

                # ---- B: l1norm scales + labels + direction d ----
                if "dchain" in skip:
                    diff0 = smt.tile([128, 128], F32, tag="diff", bufs=1)
                    nc.vector.tensor_sub(diff0, xsol, xs)
                    nc.sync.dma_start(
                        out=labels_o[s].rearrange("(p c) -> p c", p=128), in_=diff0
                    )
                    nc.gpsimd.dma_start(
                        out=preds_o[s].rearrange("(p c) -> p c", p=128), in_=pred
                    )
                    d_c = smt.tile([128, 128], BF16, tag="d_c")
                    nc.vector.memset(d_c, 0.01)
                elif not last:
                    # scale-folded direction: d' = pred + s * 3tau/(xs+tau)
                    # (s = sum_g |pred|; the true d = d'/s, and the division
                    # is deferred into the line-search alpha, which is
                    # scale-invariant up to the min-with-STEP_ALPHA).
                    # s chain runs on the colmat, in parallel with the pred
                    # transpose.
                    apred = smt.tile([128, 128], F32, tag="apred", bufs=1)
                    nc.vector.scalar_tensor_tensor(
                        out=apred, in0=pred_cm, scalar=-1.0, in1=pred_cm,
                        op0=OP.mult, op1=OP.max,
                    )
                    part8 = smt.tile([128, GPC], F32, tag="part8")
                    nc.vector.tensor_reduce(
                        out=part8,
                        in_=apred.rearrange("p (g c) -> p g c", g=GPC),
                        axis=AX.X,
                        op=OP.add,
                    )
                    s8c_ps = sg_ps.tile([128, 1], F32, tag="sg")
                    nc.tensor.matmul(
                        s8c_ps[0:GPC, :], part8, onesf, start=True, stop=True
                    )
                    s8c = smt.tile([GPC, 1], F32, tag="s8c")
                    nc.vector.tensor_scalar_max(s8c, s8c_ps[0:GPC, :], 1e-8)
                    scol_ps = sg_ps.tile([128, 1], F32, tag="sg")
                    nc.tensor.matmul(scol_ps, seg8, s8c, start=True, stop=True)
                    s_col = smt.tile([128, 1], F32, tag="s_col")
                    nc.vector.tensor_copy(s_col, scol_ps)
                    # s as a row for the alpha stage (off the d critical path)
                    s8r_ps = sg_ps.tile([1, 128], F32, tag="sgw", name="s8r_ps")
                    nc.tensor.transpose(s8r_ps[:, 0:GPC], s8c, identf[0:GPC, 0:GPC])
                    s8row = smt.tile([1, GPC], F32, tag="s8row")
                    nc.vector.tensor_copy(s8row, s8r_ps[:, 0:GPC])

                    nc.gpsimd.dma_start(
                        out=preds_o[s].rearrange("(p c) -> p c", p=128), in_=pred
                    )
                    d_bf = smt.tile([128, 128], BF16, tag="d_bf")
                    nc.vector.scalar_tensor_tensor(
                        out=d_bf, in0=recv, scalar=s_col, in1=pred,
                        op0=OP.mult, op1=OP.add,
                    )
                    if debug and s == 0:
                        d_dbg = smt.tile([128, 128], F32, tag="d_dbg")
                        nc.vector.tensor_copy(d_dbg, d_bf)
                        nc.sync.dma_start(out=dbg_d0[:], in_=d_dbg)
                    dct_ps = tr_ps.tile([128, 128], BF16, tag="tr", name="dct_ps")
                    nc.tensor.transpose(dct_ps, d_bf, identb)
                    d_c = smt.tile([128, 128], BF16, tag="d_c")
                    nc.vector.tensor_copy(d_c, dct_ps)
                elif "dchain" not in skip:
                    nc.gpsimd.dma_start(
                        out=preds_o[s].rearrange("(p c) -> p c", p=128), in_=pred
                    )

                if last:
                    continue   # einsums / line search / xs update feed only
                               # the next step's state - dead in the last step

                # ---- C: einsum1 (column form)  df[g,f] = sum_n P[g,n,f] d[g,n]
                df_ps = cm_ps.tile([128, 32], F32, tag="df_ps")
                NK1 = 2 if "e1mm" in skip else NCH
                dfb67 = smt.tile([128, 8], BF16, tag="dfb67")
                for g in [6, 7, 0, 1, 2, 3, 4, 5]:
                    for j in range(FCH):
                        for k in range(NK1):
                            nc.tensor.matmul(
                                df_ps[:, 4 * g + j : 4 * g + j + 1],
                                sbP[:, g, k, 128 * j : 128 * (j + 1)],
                                d_c[:, 16 * g + k : 16 * g + k + 1],
                                start=(k == 0),
                                stop=(k == NK1 - 1),
                            )
                    if g == 7:
                        # early evac of the DVE-path df columns so that path
                        # overlaps the remaining PE einsum work
                        nc.vector.tensor_copy(dfb67, df_ps[:, 24:32])
                dfb = smt.tile([128, 32], BF16, tag="dfb")
                nc.vector.tensor_copy(dfb, df_ps)
                if debug and s == 0:
                    dfb32 = smt.tile([128, 32], F32, tag="dfb32")
                    nc.vector.tensor_copy(dfb32, df_ps)
                    nc.sync.dma_start(out=dbg_df0[:], in_=dfb32)

                # ---- D: einsum2 (column form)  y[g,n] = sum_f PT[g,f,n] df[g,f]
                y_ps = cm_ps.tile([128, 128], F32, tag="y_ps")
                NK2 = 2 if "e2mm" in skip else NCH
                NGPE = GPC - 2   # last 2 graphs run einsum2 on DVE from resident P
                for g in range(NGPE):
                    ptts = []
                    for j in range(FCH):
                        ptt = ptp.tile([128, NMAX], BF16, tag="ptt", name="ptt")
                        if "ptdma" not in skip:
                            nc.sync.dma_start(out=ptt, in_=PT_d[g, j])
                        else:
                            nc.sync.dma_start(out=ptt[:, 0:256], in_=PT_d[g, j, :, 0:256])
                        ptts.append(ptt)
                    for k in range(NK2):
                        for j in range(FCH):
                            nc.tensor.matmul(
                                y_ps[:, 16 * g + k : 16 * g + k + 1],
                                ptts[j][:, 128 * k : 128 * (k + 1)],
                                dfb[:, 4 * g + j : 4 * g + j + 1],
                                start=(j == 0),
                                stop=(j == FCH - 1),
                            )
                # DVE path: y'[g,128k+r] = sum_f P[g,128k+r,f] * df'[g,f]
                y_dve = smt.tile([128, 2 * NCH], F32, tag="y_dve")
                for gi in range(2):
                    g = NGPE + gi
                    # df columns -> one row at partition 0 -> broadcast tile
                    df_row = smt.tile([1, F], BF16, tag="df_row", bufs=2)
                    for j in range(FCH):
                        dfr_ps = tr_ps.tile(
                            [128, 128], BF16, tag="tr", name=f"dfr{g}_{j}"
                        )
                        nc.tensor.transpose(
                            dfr_ps[0:1, :],
                            dfb67[:, 4 * gi + j : 4 * gi + j + 1],
                            identb,
                        )
                        nc.vector.tensor_copy(
                            df_row[:, 128 * j : 128 * (j + 1)], dfr_ps[0:1, :]
                        )
                    bc_ps = mlp_ps.tile([128, 512], F32, tag="hpsum", name=f"bcps{g}")
                    nc.tensor.matmul(bc_ps, onesb, df_row, start=True, stop=True)
                    df_bc = smt.tile([128, F], BF16, tag="df_bc", bufs=1)
                    nc.vector.tensor_copy(df_bc, bc_ps)
                    junk = smt.tile([128, F], BF16, tag="e2junk", bufs=1)
                    for k in range(NK2):
                        nc.vector.tensor_mul(junk, sbP[:, g, k, :], df_bc)
                        nc.vector.tensor_reduce(
                            out=y_dve[:, 16 * gi + k : 16 * gi + k + 1],
                            in_=junk, axis=AX.X, op=OP.add,
                        )
                y_cm = smt.tile([128, 128], BF16, tag="y_cm")
                nc.vector.tensor_copy(y_cm[:, 0 : 16 * NGPE], y_ps[:, 0 : 16 * NGPE])
                nc.vector.tensor_copy(y_cm[:, 16 * NGPE : 128], y_dve)
                yT_ps = tr_ps.tile([128, 128], BF16, tag="tr", name="yT_ps")
                nc.tensor.transpose(yT_ps, y_cm, identb)
                y_pm = smt.tile([128, 128], BF16, tag="y_pm", bufs=2)
                nc.vector.tensor_copy(y_pm, yT_ps)
                if debug and s == 0:
                    y_dbg = smt.tile([128, 128], F32, tag="y_dbg")
                    nc.vector.tensor_copy(y_dbg, y_pm)
                    nc.sync.dma_start(out=dbg_y0[:], in_=y_dbg)

                # ---- E: line search + state update ----
                if "alpha" in skip:
                    acol0 = smt.tile([128, 1], F32, tag="acol")
                    nc.vector.memset(acol0, 0.05)
                    ay0 = smt.tile([128, 128], F32, tag="ay", bufs=1)
                    nc.vector.tensor_scalar(
                        out=ay0, in0=y_pm, scalar1=acol0, scalar2=None, op0=OP.mult
                    )
                    nc.vector.tensor_add(xs, xs, ay0)
                    continue
                q = smt.tile([128, 128], F32, tag="q", bufs=1)
                nc.vector.tensor_scalar(
                    out=q, in0=y_pm, scalar1=-1.0, scalar2=1e-30,
                    op0=OP.mult, op1=OP.max,
                )
                nc.vector.reciprocal(q, q)
                stp = smt.tile([128, 128], F32, tag="stp", bufs=1)
                nc.vector.tensor_mul(stp, xs, q)
                smin = smt.tile([128, 1], F32, tag="smin")
                nc.vector.tensor_reduce(out=smin, in_=stp, axis=AX.X, op=OP.min)
                # per-graph min: transpose -> row -> seg-min -> back to column
                smin_ps = sg_ps.tile([1, 128], F32, tag="sgw")
                nc.tensor.transpose(smin_ps, smin, identf)
                smin_row = smt.tile([1, 128], F32, tag="smin_row")
                nc.vector.tensor_copy(smin_row, smin_ps)
                amin_row = smt.tile([1, GPC], F32, tag="amin_row")
                nc.vector.tensor_reduce(
                    out=amin_row,
                    in_=smin_row.rearrange("o (g b) -> o g b", g=GPC),
                    axis=AX.X,
                    op=OP.min,
                )
                if "dchain" not in skip:
                    nc.vector.tensor_mul(amin_row, amin_row, s8row)
                nc.vector.tensor_scalar(
                    out=amin_row, in0=amin_row, scalar1=float(STEP_ALPHA),
                    scalar2=0.995, op0=OP.min, op1=OP.mult,
                )
                if "dchain" not in skip:
                    s8inv = smt.tile([1, GPC], F32, tag="s8inv")
                    nc.vector.reciprocal(s8inv, s8row)
                    nc.vector.tensor_mul(amin_row, amin_row, s8inv)
                if debug:
                    nc.sync.dma_start(out=dbg_alpha[s], in_=amin_row)
                a8_ps = sg_ps.tile([GPC, 1], F32, tag="sg")
                nc.tensor.transpose(a8_ps, amin_row, identf[0:1, 0:1])
                a8 = smt.tile([GPC, 1], F32, tag="a8")
                nc.vector.tensor_copy(a8, a8_ps)
                acol_ps = sg_ps.tile([128, 1], F32, tag="sg")
                nc.tensor.matmul(acol_ps, seg8, a8, start=True, stop=True)
                acol = smt.tile([128, 1], F32, tag="acol")
                nc.vector.tensor_copy(acol, acol_ps)

                ay = smt.tile([128, 128], F32, tag="ay", bufs=1)
                nc.vector.tensor_scalar(
                    out=ay, in0=y_pm, scalar1=acol, scalar2=None, op0=OP.mult
                )
                nc.vector.tensor_add(xs, xs, ay)

    _split_sync_waits(nc, maxw=1)
    return nc


def _seg_mats():
    seg = np.zeros((128, 128), np.float32)
    for g in range(GPC):
        seg[16 * g : 16 * g + 16, 16 * g : 16 * g + 16] = 1.0
    seg8 = np.zeros((GPC, 128), np.float32)
    for g in range(GPC):
        seg8[g, 16 * g : 16 * g + 16] = 1.0
    return seg, seg8


def _prep_core_inputs(core, proj, x_start, x_solution, node_feat, W1, b1, W2, b2):
    g0 = core * GPC
    n0 = core * NPC
    Pc = proj[g0 : g0 + GPC]  # [8, 2048, 512] f32
    P_bf = np.ascontiguousarray(
        Pc.reshape(GPC, NCH, 128, F).transpose(2, 0, 1, 3)
    ).astype(BF)
    PT_bf = np.ascontiguousarray(Pc.transpose(0, 2, 1)).reshape(
        GPC, FCH, 128, NMAX
    ).astype(BF)
    nfTp = np.empty((NFEAT + 2, NPC), BF)
    nfTp[0:NFEAT] = np.ascontiguousarray(node_feat[n0 : n0 + NPC].T).astype(BF)
    nfTp[NFEAT] = np.ones((NPC,), BF)
    nfTp[NFEAT + 1] = np.zeros((NPC,), BF)  # xs row, written on device
    w1a = np.empty((NFEAT + 2, HID), BF)
    w1a[0:NFEAT] = W1[0:NFEAT].astype(BF)
    w1a[NFEAT] = b1.astype(BF)
    w1a[NFEAT + 1] = W1[NFEAT].astype(BF)
    seg, seg8 = _seg_mats()
    return {
        "P": P_bf,
        "PT": PT_bf,
        "nfTp": nfTp,
        "xs0": x_start[n0 : n0 + NPC].reshape(128, 128).astype(np.float32),
        "xsol": x_solution[n0 : n0 + NPC].reshape(128, 128).astype(np.float32),
        "w1a": w1a,
        "w2": W2.reshape(HID, 1).astype(BF),
        "b2": b2.reshape(1, 1).astype(np.float32),
        "seg": seg,
        "seg8": seg8,
    }


def _numpy_fallback(x_start, x_solution, node_feat, proj_matrix, W1, b1, W2, b2, batch):
    """General (ragged) reference implementation in numpy, used only if
    vals_batch is not the expected equal-size pattern."""
    nb = proj_matrix.shape[0]
    batch = batch.astype(np.int64)
    counts = np.bincount(batch, minlength=nb)
    offsets = np.cumsum(counts) - counts
    pos = np.arange(batch.shape[0]) - offsets[batch]

    def l1norm(x):
        s = np.zeros(nb, x.dtype)
        np.add.at(s, batch, np.abs(x))
        return x / np.clip(s, 1e-8, None)[batch]

    def to_dense(x):
        dense = np.zeros((nb, NMAX), x.dtype)
        m = pos < NMAX
        dense[batch[m], pos[m]] = x[m]
        return dense

    def line_search(x, dvec):
        neg = dvec < 0
        step = np.where(neg, x / np.where(neg, -dvec, 1.0), STEP_ALPHA)
        a = np.full(nb, np.inf, step.dtype)
        np.minimum.at(a, batch, step)
        return np.minimum(a, STEP_ALPHA)[batch]

    def gnn(x):
        h = np.concatenate([node_feat, x[:, None]], axis=-1)
        h = np.maximum(h @ W1 + b1, 0.0)
        return (h @ W2 + b2)[:, 0]

    tau = 0.01
    xs = x_start.astype(np.float32)
    preds, labels = [], []
    for _ in range(NUM_STEPS):
        pred = gnn(xs)
        preds.append(pred)
        labels.append(l1norm(x_solution - xs))
        p = l1norm(pred)
        direction = p + 3.0 * tau / (xs + tau)
        tau = max(tau * 0.5, 1e-5)
        d_dense = to_dense(direction)
        df = np.einsum("bnf,bn->bf", proj_matrix, d_dense)
        proj_dense = np.einsum("bnf,bf->bn", proj_matrix, df)
        proj_flat = proj_dense[batch, np.minimum(pos, NMAX - 1)]
        proj_flat = np.where(pos < NMAX, proj_flat, 0.0)
        alpha = line_search(xs, proj_flat) * 0.995
        xs = xs + alpha * proj_flat
    return np.stack(preds, 1).astype(np.float32), np.stack(labels, 1).astype(np.float32)


def run_on_hw(inputs_list, debug=False):
    key = "dbg" if debug else "plain"
    if key not in _COMPILED:
        _COMPILED[key] = build_nc(debug=debug)
    nc = _COMPILED[key]
    return run_bass_kernel_spmd(nc, inputs_list, list(range(NCORES))).results


def kernel(x_start, x_solution, node_feat, proj_matrix, W1, b1, W2, b2, vals_batch):
    expected = np.repeat(np.arange(B, dtype=np.int64), NMAX)
    vb = np.asarray(vals_batch)
    if vb.shape != expected.shape or not np.array_equal(
        vb.astype(np.int64), expected
    ):
        return _numpy_fallback(
            np.asarray(x_start, np.float32),
            np.asarray(x_solution, np.float32),
            np.asarray(node_feat, np.float32),
            np.asarray(proj_matrix, np.float32),
            np.asarray(W1, np.float32),
            np.asarray(b1, np.float32),
            np.asarray(W2, np.float32),
            np.asarray(b2, np.float32),
            vb,
        )

    x_start = np.asarray(x_start, np.float32)
    x_solution = np.asarray(x_solution, np.float32)
    node_feat = np.asarray(node_feat, np.float32)
    proj_matrix = np.asarray(proj_matrix, np.float32)
    W1 = np.asarray(W1, np.float32)
    b1 = np.asarray(b1, np.float32)
    W2 = np.asarray(W2, np.float32)
    b2 = np.asarray(b2, np.float32)

    ins = [
        _prep_core_inputs(c, proj_matrix, x_start, x_solution, node_feat, W1, b1, W2, b2)
        for c in range(NCORES)
    ]
    results = run_on_hw(ins)
    preds = np.concatenate(
        [results[c]["preds"].T for c in range(NCORES)], axis=0
    ).astype(np.float32)
    labels = np.concatenate(
        [results[c]["labels"].T for c in range(NCORES)], axis=0
    ).astype(np.float32)
    return preds, labels
